# revision 1
# baseline (speedup 1.0000x reference)
"""Trainium2 Bass kernel for nn_GatFeatDecoder (GAT-style decoder).

Reference computation per batch b (B=16, W=64, K=256, E=128, O=64):
    v = x[b].T                               (K, W)
    l = v @ W1.T ; r = v @ W2.T              (K, E) each
    e[i,j]  = sum_e a_e * LeakyReLU(l[i,e] + r[j,e] + lin_b[e]) + bias_kk[i,j]
    attn    = softmax_j(e)
    h       = sigmoid(attn @ v)              (K, W)
    out[b]  = h.T @ fc_w.T + fc_b            (W, O)

Kernel strategy (data-parallel, 2 batches per core on 8 cores, no
collectives):
  * Fold (1-alpha)*|a| into W1/W2/lin_b rows => z~ = (1-alpha)|a| z, and
    sum_e a_e*LeakyReLU(z) = sum_e sgn_e relu(z~) + alpha' * sum_e sgn_e z~
    with alpha' = alpha/(1-alpha).  The per-i part of the linear term is
    constant across j and cancels in softmax; the per-j part is
    srb_j = alpha' * (sgn . rtb[:,j]), computed on device with one matmul.
  * relu tiles T^J[e, i] = relu(lt[e,i] + rtb[e,J]) are produced per key
    node J with one DVE tensor_scalar (add + max0, bf16 4x mode) or one
    ACT activation (Relu with per-partition bias) - work split between
    the two engines.
  * The e-contraction sum_e sgn_e T^J[e,i] is one M=1 matmul per J with
    the +-1 vector as stationary weights, 4-way column-tiled so four J
    streams run concurrently on the PE array.  Rows land in PSUM at
    partitions {0,32,64,96} x 2 free halves; one DMA per 8 J's scatters
    them into score^T tiles [j, i] in SBUF (identity j-order).
  * softmax without per-row max (logits bounded): P^T = exp(score^T +
    srb_j + bias_kk^T).  attn@v and the row-sum denominator come from one
    matmul per (i-half, j-half) with lhsT = P^T and rhs = [v | 2.0]; then
    h = sigmoid(num/den) = 0.5*(tanh(num * (0.5/den)) + 1), realized as
    ACT Tanh with per-partition scale = reciprocal(2*sum exp), with the
    0.5/0.5 affine folded into the fc weights/bias on the host.
  * fc: out^T[o,w] = sum_k (0.5 fc_w)^T t + (fc_b + 0.5 sum_k fc_w).
"""

import numpy as np

import concourse.bass as bass
import concourse.bacc as bacc
import concourse.tile as tile
from concourse import mybir
from concourse.bass_utils import run_bass_kernel_spmd

ALPHA = 0.2
B, Wn, K, E, O = 16, 64, 256, 128, 64
N_CORES = 8
BPC = B // N_CORES  # batches per core

FP32 = mybir.dt.float32
BF16 = mybir.dt.bfloat16
AF = mybir.ActivationFunctionType
ALU = mybir.AluOpType

# fraction of relu-tile generation sent to DVE (rest to ACT)
DVE_SHARE_MOD = 4  # J % DVE_SHARE_MOD == DVE_SHARE_MOD-1 -> ACT


def _build_program():
    nc = bacc.Bacc("TRN2", target_bir_lowering=False, debug=False,
                   num_devices=N_CORES)

    # ---- per-core DRAM I/O ----
    d_x = nc.dram_tensor("xin", [BPC, Wn, K], FP32, kind="ExternalInput")
    d_xto2 = nc.dram_tensor("xto2", [BPC, K, Wn + 1], FP32, kind="ExternalInput")
    d_w1at = nc.dram_tensor("w1at", [Wn, E], FP32, kind="ExternalInput")
    d_w2bt = nc.dram_tensor("w2bt", [Wn + 1, E], FP32, kind="ExternalInput")
    d_sgnw = nc.dram_tensor("sgnw", [E, 1024], FP32, kind="ExternalInput")
    d_asgn = nc.dram_tensor("asgnv", [E, 1], FP32, kind="ExternalInput")
    d_bkkt = nc.dram_tensor("bkkt", [K, K], FP32, kind="ExternalInput")
    d_fcw2t = nc.dram_tensor("fcw2t", [K, O], FP32, kind="ExternalInput")
    d_fcb2 = nc.dram_tensor("fcb2", [O, 1], FP32, kind="ExternalInput")
    d_out = nc.dram_tensor("outp", [BPC, O, Wn], FP32, kind="ExternalOutput")

    with tile.TileContext(nc) as tc:
        with (
            tc.tile_pool(name="consts", bufs=1) as consts,
            tc.tile_pool(name="setup", bufs=2) as setup,
            tc.tile_pool(name="trelu", bufs=16) as trelu,
            tc.tile_pool(name="etiles", bufs=4) as etiles,
            tc.tile_pool(name="small", bufs=8) as small,
            tc.tile_pool(name="psA", bufs=1, space="PSUM") as psA,
            tc.tile_pool(name="psS", bufs=2, space="PSUM") as psS,
            tc.tile_pool(name="psV", bufs=1, space="PSUM") as psV,
            tc.tile_pool(name="psF", bufs=1, space="PSUM") as psF,
            tc.tile_pool(name="psR", bufs=1, space="PSUM") as psR,
        ):
            # ---------- load constants ----------
            w1at = consts.tile([Wn, E], FP32, tag="w1at")
            nc.gpsimd.dma_start(out=w1at[:], in_=d_w1at.ap())
            w2bt = consts.tile([Wn + 1, E], FP32, tag="w2bt")
            nc.gpsimd.dma_start(out=w2bt[:], in_=d_w2bt.ap())
            sgnw_f = consts.tile([E, 1024], FP32, tag="sgnwf")
            nc.gpsimd.dma_start(out=sgnw_f[:], in_=d_sgnw.ap())
            asgn_f = consts.tile([E, 1], FP32, tag="asgnf")
            nc.gpsimd.dma_start(out=asgn_f[:], in_=d_asgn.ap())
            bkkt = [consts.tile([K // 2, K], FP32, tag=f"bkkt{t}", name=f"bkkt{t}") for t in range(2)]
            for t in range(2):
                nc.gpsimd.dma_start(out=bkkt[t][:], in_=d_bkkt.ap()[128 * t:128 * t + 128, :])
            fcw2t = [consts.tile([K // 2, O], FP32, tag=f"fcw2t{h}",
                                 name=f"fcw2t{h}") for h in range(2)]
            for h in range(2):
                nc.gpsimd.dma_start(out=fcw2t[h][:],
                                  in_=d_fcw2t.ap()[128 * h:128 * h + 128, :])
            fcb2 = consts.tile([O, 1], FP32, tag="fcb2")
            nc.gpsimd.dma_start(out=fcb2[:], in_=d_fcb2.ap())

            # bf16 casts (also funnels PE inputs through DVE so PE
            # instructions wait on a single semaphore)
            w1at_b = consts.tile([Wn, E], BF16, tag="w1atb")
            nc.vector.tensor_copy(w1at_b[:], w1at[:])
            w2bt_b = consts.tile([Wn + 1, E], BF16, tag="w2btb")
            nc.vector.tensor_copy(w2bt_b[:], w2bt[:])
            sgnw_b = consts.tile([E, 1024], BF16, tag="sgnwb")
            nc.vector.tensor_copy(sgnw_b[:], sgnw_f[:])
            asgn_b = consts.tile([E, 1], BF16, tag="asgnb")
            nc.vector.tensor_copy(asgn_b[:], asgn_f[:])
            fcw2t_b = [consts.tile([K // 2, O], BF16, tag=f"fcw2tb{h}",
                                   name=f"fcw2tb{h}") for h in range(2)]
            for h in range(2):
                nc.vector.tensor_copy(fcw2t_b[h][:], fcw2t[h][:])

            for b in range(BPC):
                # ---------- setup: lt / rtb projections ----------
                xb = setup.tile([Wn + 1, K], FP32, tag="xb")
                nc.gpsimd.dma_start(out=xb[0:Wn, :], in_=d_x.ap()[b])
                nc.vector.memset(xb[Wn:Wn + 1, :], 1.0)
                xb_b = setup.tile([Wn + 1, K], BF16, tag="xb_b")
                nc.vector.tensor_copy(xb_b[:], xb[:])

                ps_lt = psA.tile([E, K], FP32, tag="ps_lt")
                nc.tensor.matmul(ps_lt[:], w1at_b[:], xb_b[0:Wn, :],
                                 start=True, stop=True)
                ps_rt = psA.tile([E, K], FP32, tag="ps_rt")
                nc.tensor.matmul(ps_rt[:], w2bt_b[:], xb_b[:],
                                 start=True, stop=True)

                lt_b = setup.tile([E, K], BF16, tag="lt_b")
                nc.scalar.copy(lt_b[:], ps_lt[:])
                rtb_f = setup.tile([E, K], FP32, tag="rtb_f")
                nc.scalar.copy(rtb_f[:], ps_rt[:])
                rtb_b = setup.tile([E, K], BF16, tag="rtb_b")
                nc.vector.tensor_copy(rtb_b[:], ps_rt[:])

                # srb[j] = alpha' * sum_e sgn_e rtb[e, j]  (column, per j-half)
                ps_srb = psR.tile([K // 2, 2], FP32, tag="ps_srb")
                for t in range(2):
                    nc.tensor.matmul(ps_srb[:, t:t + 1],
                                     rtb_b[:, 128 * t:128 * t + 128],
                                     asgn_b[:], start=True, stop=True)
                srb = small.tile([K // 2, 2], FP32, tag="srb")
                nc.vector.tensor_copy(srb[:], ps_srb[:])

                # xto2 (rhs for attn@v), cast to bf16 per j-half
                xto = [small.tile([K // 2, Wn + 1], BF16, tag=f"xto{h}", name=f"xto{h}")
                       for h in range(2)]
                for h in range(2):
                    xf = small.tile([K // 2, Wn + 1], FP32, tag=f"xtof{h}")
                    nc.gpsimd.dma_start(
                        out=xf[:], in_=d_xto2.ap()[b, 128 * h:128 * h + 128, :])
                    nc.vector.tensor_copy(xto[h][:], xf[:])

                # ---------- relu-gen + scatter-contraction ----------
                # scores^T tile [j-half=128, i=256] accumulates directly in
                # PSUM: the matmul for J uses a [128, 32] weight tile whose
                # only nonzero column (at index m = J%32) is the sign
                # vector, so the score row lands on partition 32g + m.
                pT = [etiles.tile([K // 2, K], BF16, tag=f"pT{t}", name=f"pT{t}") for t in range(2)]
                for jh in range(2):
                    ps_sc = psS.tile([128, K], FP32, tag="ps_sc")
                    for m in range(32):
                        for g in range(4):
                            J = 128 * jh + 32 * g + m
                            tj = trelu.tile([E, K], BF16, tag="tj")
                            if m % 5 == 2:
                                nc.scalar.activation(
                                    tj[:], lt_b[:], AF.Relu,
                                    bias=rtb_f[:, J:J + 1], scale=1.0)
                            else:
                                nc.vector.tensor_scalar(
                                    out=tj[:], in0=lt_b[:],
                                    scalar1=rtb_f[:, J:J + 1], scalar2=0.0,
                                    op0=ALU.add, op1=ALU.max)
                            nc.tensor.matmul(
                                ps_sc[32 * g:32 * g + 32, :],
                                sgnw_b[:, 32 * m:32 * m + 32], tj[:],
                                start=(m == 0), stop=(m == 31),
                                tile_position=(0, 32 * g),
                                skip_group_check=True)
                    # logits -> P^T = exp(S + srb_j + bias_kk^T)
                    et = etiles.tile([K // 2, K], FP32, tag="et")
                    nc.vector.scalar_tensor_tensor(
                        out=et[:], in0=ps_sc[:], scalar=srb[:, jh:jh + 1],
                        in1=bkkt[jh][:], op0=ALU.add, op1=ALU.add)
                    nc.scalar.activation(pT[jh][:], et[:], AF.Exp)

                # ---------- attn @ [v | 2] ----------
                t_ih = []
                for ih in range(2):
                    ps_num = psV.tile([K // 2, Wn + 1], FP32, tag="ps_num")
                    for jh in range(2):
                        nc.tensor.matmul(
                            ps_num[:], pT[jh][:, 128 * ih:128 * ih + 128],
                            xto[jh][:], start=(jh == 0), stop=(jh == 1))
                    rcol = small.tile([K // 2, 1], FP32, tag=f"rcol{ih}")
                    nc.vector.reciprocal(rcol[:], ps_num[:, Wn:Wn + 1])
                    tt = small.tile([K // 2, Wn], BF16, tag=f"tt{ih}")
                    nc.scalar.activation(tt[:], ps_num[:, 0:Wn], AF.Tanh,
                                         scale=rcol[:])
                    t_ih.append(tt)

                # ---------- fc: out^T = (0.5 fc_w)^T t + fcb2 ----------
                ps_o = psF.tile([O, Wn], FP32, tag="ps_o")
                for ih in range(2):
                    nc.tensor.matmul(ps_o[:], fcw2t_b[ih][:],
                                     t_ih[ih][:], start=(ih == 0), stop=(ih == 1))
                ot = small.tile([O, Wn], FP32, tag="ot")
                nc.scalar.activation(ot[:], ps_o[:], AF.Identity, bias=fcb2[:])
                nc.gpsimd.dma_start(out=d_out.ap()[b], in_=ot[:])

    nc.compile()
    return nc


_NC_CACHE = {}


def _get_program():
    if "nc" not in _NC_CACHE:
        _NC_CACHE["nc"] = _build_program()
    return _NC_CACHE["nc"]


def _host_prep(x, lin_w, lin_b, a, bias_kk, fc_w, fc_b):
    f32 = np.float32
    x = np.ascontiguousarray(x, f32)
    aa = (np.abs(a) * (1.0 - ALPHA)).astype(f32)
    sgn = np.sign(a).astype(f32)
    w1at = np.ascontiguousarray((lin_w[:, :Wn] * aa[:, None]).T, f32)
    w2t = (lin_w[:, Wn:] * aa[:, None]).T
    bt = (lin_b * aa)[None, :]
    w2bt = np.ascontiguousarray(np.concatenate([w2t, bt], 0), f32)
    xto2 = np.concatenate(
        [np.transpose(x, (0, 2, 1)),
         np.full((B, K, 1), 2.0, f32)], axis=2)
    xto2 = np.ascontiguousarray(xto2, f32)
    bkkt = np.ascontiguousarray(bias_kk.T, f32)
    fcw2t = np.ascontiguousarray((0.5 * fc_w).T, f32)
    fcb2 = np.ascontiguousarray(
        (fc_b + 0.5 * fc_w.sum(1)).reshape(O, 1), f32)
    sgnw = np.zeros((E, 1024), f32)
    for m in range(32):
        sgnw[:, 32 * m + m] = sgn
    shared = dict(w1at=w1at, w2bt=w2bt, sgnw=sgnw,
                  asgnv=np.ascontiguousarray((0.25 * sgn).reshape(E, 1)),
                  bkkt=bkkt, fcw2t=fcw2t, fcb2=fcb2)
    in_maps = []
    for c in range(N_CORES):
        m = dict(shared)
        m["xin"] = np.ascontiguousarray(x[BPC * c:BPC * (c + 1)])
        m["xto2"] = np.ascontiguousarray(xto2[BPC * c:BPC * (c + 1)])
        in_maps.append(m)
    return in_maps


def kernel(x, lin_w, lin_b, a, bias_kk, fc_w, fc_b, _trace=False):
    nc = _get_program()
    in_maps = _host_prep(np.asarray(x), np.asarray(lin_w), np.asarray(lin_b),
                         np.asarray(a), np.asarray(bias_kk),
                         np.asarray(fc_w), np.asarray(fc_b))
    res = run_bass_kernel_spmd(nc, in_maps, list(range(N_CORES)),
                               trace=_trace)
    out = np.empty((B, Wn, O), np.float32)
    for c in range(N_CORES):
        o = res.results[c]["outp"]          # (BPC, O, Wn)
        for i in range(BPC):
            out[BPC * c + i] = o[i].T
    if _trace:
        return out, res
    return out



# revision 5
# speedup vs baseline: 1.3124x; 1.3124x over previous
"""Trainium2 Bass kernel for nn_GatFeatDecoder (GAT-style decoder).

Reference computation per batch b (B=16, W=64, K=256, E=128, O=64):
    v = x[b].T                               (K, W)
    l = v @ W1.T ; r = v @ W2.T              (K, E) each
    e[i,j]  = sum_e a_e * LeakyReLU(l[i,e] + r[j,e] + lin_b[e]) + bias_kk[i,j]
    attn    = softmax_j(e)
    h       = sigmoid(attn @ v)              (K, W)
    out[b]  = h.T @ fc_w.T + fc_b            (W, O)

Kernel strategy (data-parallel, 2 batches per core on 8 cores, no
collectives):
  * Fold (1-alpha)*|a| into W1/W2/lin_b rows => z~ = (1-alpha)|a| z, and
    sum_e a_e*LeakyReLU(z) = sum_e sgn_e relu(z~) + alpha' * sum_e sgn_e z~
    with alpha' = alpha/(1-alpha).  The per-i part of the linear term is
    constant across j and cancels in softmax; the per-j part
    srb_j = alpha' * (sgn . rtb[:,j]) is folded MULTIPLICATIVELY into the
    attention rhs: exp(S + srb_j) = exp(S) * exp(srb_j), so the rows of
    [v | 2] are scaled by exp(srb_j) instead of adding srb pre-exp.
  * Relu tiles are generated PER QUERY NODE i: T_i[e, j] =
    relu(rtb[e, j] + lt[e, i]) with one fused tensor_scalar (DVE, 4x bf16
    mode), one ACT activation (Relu + per-partition bias), or one GPSIMD
    tensor_scalar - work split across all three engines by a static
    greedy load balancer (DVE ~127ns, ACT ~398ns, Pool ~451ns per tile).
  * The e-contraction uses the tile as STATIONARY weights: per (i, jh)
    one 1-row matmul with moving operand sgn [E,1] writes column i of
    the PSUM score tile S^T[j-half, i] directly in the [j, i] layout the
    softmax needs - no transpose, ~4ns of PE time per matmul.
  * bias_kk^T is pre-accumulated into the PSUM score tiles with one
    identity-weight matmul per (jh) before the column matmuls.
  * softmax without per-row max (logits bounded): P^T = exp(S^T).
    attn@v and the row-sum denominator come from one matmul per
    (i-half, j-half) with lhsT = P^T and rhs = exp(srb)*[v | 2]; then
    h = sigmoid(num/den) = 0.5*(tanh(num * (0.5/den)) + 1), realized as
    ACT Tanh with per-partition scale = reciprocal(2*sum exp), with the
    0.5/0.5 affine folded into the fc weights/bias on the host.
  * fc: out^T[o,w] = (0.5 fc_w)^T t + (fc_b + 0.5 sum_k fc_w).
"""

import numpy as np
import ml_dtypes

import concourse.bass as bass
import concourse.bacc as bacc
import concourse.tile as tile
from concourse import mybir
from concourse.bass_utils import run_bass_kernel_spmd

ALPHA = 0.2
B, Wn, K, E, O = 16, 64, 256, 128, 64
N_CORES = 8
BPC = B // N_CORES  # batches per core

FP32 = mybir.dt.float32
BF16 = mybir.dt.bfloat16
AF = mybir.ActivationFunctionType
ALU = mybir.AluOpType
NP_BF16 = ml_dtypes.bfloat16


def _tile_engine_schedule():
    """Static engine assignment for the 256 relu tiles of one batch.

    Greedy finish-time balancing with initial loads covering each
    engine's fixed per-batch work (copies/exp/tanh on ACT, casts and
    reciprocals on DVE, DMA triggers on Pool).
    """
    loads = {"D": 800.0, "A": 2280.0, "P": 500.0}
    cost = {"D": 127.0, "A": 398.0, "P": 451.0}
    sched = []
    for _ in range(K):
        e = min(loads, key=lambda k: loads[k] + cost[k])
        loads[e] += cost[e]
        sched.append(e)
    return sched


_SCHED = _tile_engine_schedule()


def _build_program():
    nc = bacc.Bacc("TRN2", target_bir_lowering=False, debug=False,
                   num_devices=N_CORES)

    # ---- per-core DRAM I/O ----
    d_xb = nc.dram_tensor("xb", [BPC, Wn + 1, K], BF16, kind="ExternalInput")
    d_xto = nc.dram_tensor("xto", [BPC, K, Wn + 1], BF16, kind="ExternalInput")
    d_w1at = nc.dram_tensor("w1at", [Wn, E], BF16, kind="ExternalInput")
    d_w2bt = nc.dram_tensor("w2bt", [Wn + 1, E], BF16, kind="ExternalInput")
    d_sgn = nc.dram_tensor("sgnv", [E, 1], BF16, kind="ExternalInput")
    d_asgn = nc.dram_tensor("asgnv", [E, 1], BF16, kind="ExternalInput")
    d_bkkt = nc.dram_tensor("bkkt", [K, K], BF16, kind="ExternalInput")
    d_ident = nc.dram_tensor("ident", [128, 128], BF16, kind="ExternalInput")
    d_fcw2t = nc.dram_tensor("fcw2t", [K, O], BF16, kind="ExternalInput")
    d_fcb2 = nc.dram_tensor("fcb2", [O, 1], FP32, kind="ExternalInput")
    d_out = nc.dram_tensor("outp", [BPC, O, Wn], FP32, kind="ExternalOutput")

    with tile.TileContext(nc) as tc:
        with (
            tc.tile_pool(name="consts", bufs=1) as consts,
            tc.tile_pool(name="setup", bufs=2) as setup,
            tc.tile_pool(name="trelu", bufs=24) as trelu,
            tc.tile_pool(name="psP", bufs=1, space="PSUM") as psP,
            tc.tile_pool(name="psS", bufs=1, space="PSUM") as psS,
            tc.tile_pool(name="psM", bufs=1, space="PSUM") as psM,
        ):
            # ---------- load constants (bf16 prepared on host) ----------
            w1at = consts.tile([Wn, E], BF16, tag="w1at")
            nc.gpsimd.dma_start(out=w1at[:], in_=d_w1at.ap())
            w2bt = consts.tile([Wn + 1, E], BF16, tag="w2bt")
            nc.gpsimd.dma_start(out=w2bt[:], in_=d_w2bt.ap())
            sgn_b = consts.tile([E, 1], BF16, tag="sgn")
            nc.gpsimd.dma_start(out=sgn_b[:], in_=d_sgn.ap())
            asgn_b = consts.tile([E, 1], BF16, tag="asgn")
            nc.gpsimd.dma_start(out=asgn_b[:], in_=d_asgn.ap())
            bkkt = [consts.tile([K // 2, K], BF16, tag=f"bkkt{t}",
                                name=f"bkkt{t}") for t in range(2)]
            for t in range(2):
                nc.gpsimd.dma_start(out=bkkt[t][:],
                                    in_=d_bkkt.ap()[128 * t:128 * t + 128, :])
            ident = consts.tile([128, 128], BF16, tag="ident")
            nc.gpsimd.dma_start(out=ident[:], in_=d_ident.ap())
            fcw2t = [consts.tile([K // 2, O], BF16, tag=f"fcw2t{h}",
                                 name=f"fcw2t{h}") for h in range(2)]
            for h in range(2):
                nc.gpsimd.dma_start(out=fcw2t[h][:],
                                    in_=d_fcw2t.ap()[128 * h:128 * h + 128, :])
            fcb2 = consts.tile([O, 1], FP32, tag="fcb2")
            nc.gpsimd.dma_start(out=fcb2[:], in_=d_fcb2.ap())

            for b in range(BPC):
                # ---------- setup: projections, scalars, rhs ----------
                xb_b = setup.tile([Wn + 1, K], BF16, tag="xb_b")
                nc.gpsimd.dma_start(out=xb_b[:], in_=d_xb.ap()[b])
                xto_b = [setup.tile([K // 2, Wn + 1], BF16, tag=f"xto{h}",
                                    name=f"xto{h}") for h in range(2)]
                for h in range(2):
                    nc.gpsimd.dma_start(
                        out=xto_b[h][:],
                        in_=d_xto.ap()[b, 128 * h:128 * h + 128, :])

                ps_lt = psP.tile([E, K], FP32, tag="ps_lt")
                nc.tensor.matmul(ps_lt[:], w1at[:], xb_b[0:Wn, :],
                                 start=True, stop=True)
                ps_rt = psP.tile([E, K], FP32, tag="ps_rt")
                nc.tensor.matmul(ps_rt[:], w2bt[:], xb_b[:],
                                 start=True, stop=True)

                # lt columns feed the per-i tile ops as scalars (SBUF f32)
                lt_f = setup.tile([E, K], FP32, tag="lt_f")
                nc.scalar.copy(lt_f[:], ps_lt[:])
                rtb_b = setup.tile([E, K], BF16, tag="rtb_b")
                nc.vector.tensor_copy(rtb_b[:], ps_rt[:])

                # srb_j = alpha' * sum_e sgn_e rtb[e, j]; fold exp(srb)
                # into the attention rhs rows
                xto_s = []
                for jh in range(2):
                    ps_srb = psM.tile([K // 2, 1], FP32, tag="srb")
                    nc.tensor.matmul(ps_srb[:],
                                     rtb_b[:, 128 * jh:128 * jh + 128],
                                     asgn_b[:], start=True, stop=True)
                    esrb = setup.tile([K // 2, 1], FP32, tag=f"esrb{jh}",
                                      name=f"esrb{jh}")
                    nc.scalar.activation(esrb[:], ps_srb[:], AF.Exp)
                    xs = setup.tile([K // 2, Wn + 1], BF16, tag=f"xts{jh}",
                                    name=f"xts{jh}")
                    nc.vector.tensor_scalar(out=xs[:], in0=xto_b[jh][:],
                                            scalar1=esrb[:], scalar2=None,
                                            op0=ALU.mult)
                    xto_s.append(xs)

                # ---------- scores: S^T[j, i] in PSUM ----------
                ps_sc = [psS.tile([K // 2, K], FP32, tag=f"psS{jh}",
                                  name=f"psS{jh}") for jh in range(2)]
                for jh in range(2):
                    # pre-accumulate bias_kk^T block via identity weights
                    nc.tensor.matmul(ps_sc[jh][:], ident[:], bkkt[jh][:],
                                     start=True, stop=False,
                                     skip_group_check=True)
                for i in range(K):
                    tj = trelu.tile([E, K], BF16, tag="tj")
                    eng = _SCHED[i]
                    if eng == "D":
                        nc.vector.tensor_scalar(
                            out=tj[:], in0=rtb_b[:],
                            scalar1=lt_f[:, i:i + 1], scalar2=0.0,
                            op0=ALU.add, op1=ALU.max)
                    elif eng == "A":
                        nc.scalar.activation(
                            tj[:], rtb_b[:], AF.Relu,
                            bias=lt_f[:, i:i + 1], scale=1.0)
                    else:
                        nc.gpsimd.tensor_scalar(
                            out=tj[:], in0=rtb_b[:],
                            scalar1=lt_f[:, i:i + 1], scalar2=0.0,
                            op0=ALU.add, op1=ALU.max)
                    for jh in range(2):
                        nc.tensor.matmul(
                            ps_sc[jh][:, i:i + 1],
                            tj[:, 128 * jh:128 * jh + 128], sgn_b[:],
                            start=False, stop=(i == K - 1),
                            skip_group_check=True)

                # ---------- softmax numerator/denominator ----------
                pT = [setup.tile([K // 2, K], BF16, tag=f"pT{jh}",
                                 name=f"pT{jh}") for jh in range(2)]
                for jh in range(2):
                    nc.scalar.activation(pT[jh][:], ps_sc[jh][:], AF.Exp)

                t_ih = []
                for ih in range(2):
                    ps_num = psM.tile([K // 2, Wn + 1], FP32, tag="ps_num")
                    for jh in range(2):
                        nc.tensor.matmul(
                            ps_num[:], pT[jh][:, 128 * ih:128 * ih + 128],
                            xto_s[jh][:], start=(jh == 0), stop=(jh == 1))
                    rcol = setup.tile([K // 2, 1], FP32, tag=f"rcol{ih}",
                                      name=f"rcol{ih}")
                    nc.vector.reciprocal(rcol[:], ps_num[:, Wn:Wn + 1])
                    tt = setup.tile([K // 2, Wn], BF16, tag=f"tt{ih}",
                                    name=f"tt{ih}")
                    nc.scalar.activation(tt[:], ps_num[:, 0:Wn], AF.Tanh,
                                         scale=rcol[:])
                    t_ih.append(tt)

                # ---------- fc: out^T = (0.5 fc_w)^T t + fcb2 ----------
                ps_o = psM.tile([O, Wn], FP32, tag="ps_o")
                for ih in range(2):
                    nc.tensor.matmul(ps_o[:], fcw2t[ih][:], t_ih[ih][:],
                                     start=(ih == 0), stop=(ih == 1))
                ot = setup.tile([O, Wn], FP32, tag="ot")
                nc.scalar.activation(ot[:], ps_o[:], AF.Identity,
                                     bias=fcb2[:])
                nc.gpsimd.dma_start(out=d_out.ap()[b], in_=ot[:])

    nc.compile()
    return nc


_NC_CACHE = {}


def _get_program():
    if "nc" not in _NC_CACHE:
        _NC_CACHE["nc"] = _build_program()
    return _NC_CACHE["nc"]


def _host_prep(x, lin_w, lin_b, a, bias_kk, fc_w, fc_b):
    f32 = np.float32
    x = np.ascontiguousarray(x, f32)
    aa = (np.abs(a) * (1.0 - ALPHA)).astype(f32)
    sgn = np.sign(a).astype(f32)
    w1at = (lin_w[:, :Wn] * aa[:, None]).T.astype(NP_BF16)
    w2t = (lin_w[:, Wn:] * aa[:, None]).T
    bt = (lin_b * aa)[None, :]
    w2bt = np.concatenate([w2t, bt], 0).astype(NP_BF16)
    xb = np.concatenate([x, np.ones((B, 1, K), f32)], axis=1)
    xto = np.concatenate(
        [np.transpose(x, (0, 2, 1)),
         np.full((B, K, 1), 2.0, f32)], axis=2)
    shared = dict(
        w1at=np.ascontiguousarray(w1at),
        w2bt=np.ascontiguousarray(w2bt),
        sgnv=np.ascontiguousarray(sgn.reshape(E, 1).astype(NP_BF16)),
        asgnv=np.ascontiguousarray(
            (0.25 * sgn).reshape(E, 1).astype(NP_BF16)),
        bkkt=np.ascontiguousarray(bias_kk.T.astype(NP_BF16)),
        ident=np.ascontiguousarray(np.eye(128, dtype=f32).astype(NP_BF16)),
        fcw2t=np.ascontiguousarray((0.5 * fc_w).T.astype(NP_BF16)),
        fcb2=np.ascontiguousarray(
            (fc_b + 0.5 * fc_w.sum(1)).reshape(O, 1).astype(f32)),
    )
    in_maps = []
    for c in range(N_CORES):
        m = dict(shared)
        m["xb"] = np.ascontiguousarray(
            xb[BPC * c:BPC * (c + 1)].astype(NP_BF16))
        m["xto"] = np.ascontiguousarray(
            xto[BPC * c:BPC * (c + 1)].astype(NP_BF16))
        in_maps.append(m)
    return in_maps


def kernel(x, lin_w, lin_b, a, bias_kk, fc_w, fc_b, _trace=False):
    nc = _get_program()
    in_maps = _host_prep(np.asarray(x), np.asarray(lin_w), np.asarray(lin_b),
                         np.asarray(a), np.asarray(bias_kk),
                         np.asarray(fc_w), np.asarray(fc_b))
    res = run_bass_kernel_spmd(nc, in_maps, list(range(N_CORES)),
                               trace=_trace)
    out = np.empty((B, Wn, O), np.float32)
    for c in range(N_CORES):
        o = res.results[c]["outp"]          # (BPC, O, Wn)
        for i in range(BPC):
            out[BPC * c + i] = o[i].T
    if _trace:
        return out, res
    return out


# revision 8
# speedup vs baseline: 1.4101x; 1.0744x over previous
"""Trainium2 Bass kernel for nn_GatFeatDecoder (GAT-style decoder).

Reference computation per batch b (B=16, W=64, K=256, E=128, O=64):
    v = x[b].T                               (K, W)
    l = v @ W1.T ; r = v @ W2.T              (K, E) each
    e[i,j]  = sum_e a_e * LeakyReLU(l[i,e] + r[j,e] + lin_b[e]) + bias_kk[i,j]
    attn    = softmax_j(e)
    h       = sigmoid(attn @ v)              (K, W)
    out[b]  = h.T @ fc_w.T + fc_b            (W, O)

Kernel strategy (data-parallel, 2 batches per core on 8 cores, no
collectives):
  * Fold (1-alpha)*|a| into W1/W2/lin_b rows => z~ = (1-alpha)|a| z, and
    sum_e a_e*LeakyReLU(z) = sum_e sgn_e relu(z~) + alpha' * sum_e sgn_e z~
    with alpha' = alpha/(1-alpha).  The per-i part of the linear term is
    constant across j and cancels in softmax; the per-j part
    srb_j = alpha' * (sgn . rtb[:,j]) is folded MULTIPLICATIVELY into the
    attention rhs: exp(S + srb_j) = exp(S) * exp(srb_j), so the rows of
    [v | 2] are scaled by exp(srb_j) instead of adding srb pre-exp.
  * Relu tiles are generated PER QUERY NODE i: T_i[e, j] =
    relu(rtb[e, j] + lt[e, i]) with one fused tensor_scalar (DVE, 4x bf16
    mode), one ACT activation (Relu + per-partition bias), or one GPSIMD
    tensor_scalar - work split across all three engines by a static
    greedy load balancer (DVE ~127ns, ACT ~398ns, Pool ~451ns per tile).
  * The e-contraction uses the tile as STATIONARY weights: per (i, jh)
    one 1-row matmul with moving operand sgn [E,1] writes column i of
    the PSUM score tile S^T[j-half, i] directly in the [j, i] layout the
    softmax needs - no transpose, ~4ns of PE time per matmul.
  * bias_kk^T is pre-accumulated into the PSUM score tiles with one
    identity-weight matmul per (jh) before the column matmuls.
  * softmax without per-row max (logits bounded): P^T = exp(S^T).
    attn@v and the row-sum denominator come from one matmul per
    (i-half, j-half) with lhsT = P^T and rhs = exp(srb)*[v | 2]; then
    h = sigmoid(num/den) = 0.5*(tanh(num * (0.5/den)) + 1), realized as
    ACT Tanh with per-partition scale = reciprocal(2*sum exp), with the
    0.5/0.5 affine folded into the fc weights/bias on the host.
  * fc: out^T[o,w] = (0.5 fc_w)^T t + (fc_b + 0.5 sum_k fc_w).
  * DMA triggers ride on SP (HWDGE, zero engine cost) except the four
    startup-critical ones, which are spread across Pool/DVE/ACT while
    those engines are still idle.  Batch 1's input DMAs + projections are
    hoisted before batch 0's tile loop so the tile stream never stalls.
"""

import numpy as np
import ml_dtypes

import concourse.bass as bass
import concourse.bacc as bacc
import concourse.tile as tile
from concourse import mybir
from concourse.bass_utils import run_bass_kernel_spmd

ALPHA = 0.2
B, Wn, K, E, O = 16, 64, 256, 128, 64
N_CORES = 8
BPC = B // N_CORES  # batches per core

FP32 = mybir.dt.float32
BF16 = mybir.dt.bfloat16
AF = mybir.ActivationFunctionType
ALU = mybir.AluOpType
NP_BF16 = ml_dtypes.bfloat16


def _tile_engine_schedule(d0, a0, p0):
    """Static engine assignment for the 256 relu tiles of one batch:
    greedy finish-time balancing, initial loads = fixed per-batch work."""
    loads = {"D": float(d0), "A": float(a0), "P": float(p0)}
    cost = {"D": 127.0, "A": 398.0, "P": 451.0}
    sched = []
    for _ in range(K):
        e = min(loads, key=lambda k: loads[k] + cost[k])
        loads[e] += cost[e]
        sched.append(e)
    return sched


# batch 0: Pool paid ~1us for the w1at SWDGE trigger; batch 1 is clean
_SCHEDS = [_tile_engine_schedule(750, 2280, 1200),
           _tile_engine_schedule(750, 2280, 150)]


def _build_program():
    nc = bacc.Bacc("TRN2", target_bir_lowering=False, debug=False,
                   num_devices=N_CORES)

    # ---- per-core DRAM I/O (bf16 prepacked on host) ----
    d_xb = nc.dram_tensor("xb", [BPC, Wn + 1, K], BF16, kind="ExternalInput")
    # xto halves packed side by side: [j, (jh, w)] -> [128, 130]
    d_xto = nc.dram_tensor("xto", [BPC, K // 2, 2 * (Wn + 1)], BF16,
                           kind="ExternalInput")
    d_w1at = nc.dram_tensor("w1at", [Wn, E], BF16, kind="ExternalInput")
    d_w2bt = nc.dram_tensor("w2bt", [Wn + 1, E], BF16, kind="ExternalInput")
    d_sa = nc.dram_tensor("sav", [E, 2], BF16, kind="ExternalInput")
    # bias_kk^T halves + identity packed: [128, 256+256+128]
    d_bki = nc.dram_tensor("bki", [128, 2 * K + 128], BF16,
                           kind="ExternalInput")
    d_fcw2t = nc.dram_tensor("fcw2t", [K // 2, 2 * O], BF16,
                             kind="ExternalInput")
    d_fcb2 = nc.dram_tensor("fcb2", [O, 1], FP32, kind="ExternalInput")
    d_out = nc.dram_tensor("outp", [BPC, O, Wn], FP32, kind="ExternalOutput")

    with tile.TileContext(nc) as tc:
        with (
            tc.tile_pool(name="consts", bufs=1) as consts,
            tc.tile_pool(name="setup", bufs=2) as setup,
            tc.tile_pool(name="trelu", bufs=28) as trelu,
            tc.tile_pool(name="psP", bufs=1, space="PSUM") as psP,
            tc.tile_pool(name="psS", bufs=1, space="PSUM") as psS,
            tc.tile_pool(name="psM", bufs=1, space="PSUM") as psM,
        ):
            # ---------- constants ----------
            # startup-critical loads on engines that are idle at t=0
            w1at = consts.tile([Wn, E], BF16, tag="w1at")
            nc.gpsimd.dma_start(out=w1at[:], in_=d_w1at.ap())
            w2bt = consts.tile([Wn + 1, E], BF16, tag="w2bt")
            nc.gpsimd.dma_start(out=w2bt[:], in_=d_w2bt.ap())
            sa = consts.tile([E, 2], BF16, tag="sa")
            nc.gpsimd.dma_start(out=sa[:], in_=d_sa.ap())
            sgn_b = sa[:, 0:1]
            asgn_b = sa[:, 1:2]
            bki = consts.tile([128, 2 * K + 128], BF16, tag="bki")
            nc.gpsimd.dma_start(out=bki[:], in_=d_bki.ap())
            bkkt = [bki[:, 0:K], bki[:, K:2 * K]]
            ident = bki[:, 2 * K:2 * K + 128]
            fcw = consts.tile([K // 2, 2 * O], BF16, tag="fcw")
            nc.gpsimd.dma_start(out=fcw[:], in_=d_fcw2t.ap())
            fcw2t = [fcw[:, 0:O], fcw[:, O:2 * O]]
            fcb2 = consts.tile([O, 1], FP32, tag="fcb2")
            nc.gpsimd.dma_start(out=fcb2[:], in_=d_fcb2.ap())

            # ---------- per-batch setup (both batches up front) ----------
            xb_b, lt_f, rtb_b, xto_s = [], [], [], []
            for b in range(BPC):
                xb = setup.tile([Wn + 1, K], BF16, tag=f"xb{b}",
                                name=f"xb{b}")
                if b == 0:
                    nc.gpsimd.dma_start(out=xb[:], in_=d_xb.ap()[b])
                else:
                    nc.gpsimd.dma_start(out=xb[:], in_=d_xb.ap()[b])
                xb_b.append(xb)
                xt = setup.tile([K // 2, 2 * (Wn + 1)], BF16, tag=f"xt{b}",
                                name=f"xt{b}")
                nc.gpsimd.dma_start(out=xt[:], in_=d_xto.ap()[b])
                xto_s.append([xt[:, 0:Wn + 1], xt[:, Wn + 1:2 * (Wn + 1)]])

            for b in range(BPC):
                ps_lt = psP.tile([E, K], FP32, tag="ps_lt")
                nc.tensor.matmul(ps_lt[:], w1at[:], xb_b[b][0:Wn, :],
                                 start=True, stop=True)
                ps_rt = psP.tile([E, K], FP32, tag="ps_rt")
                nc.tensor.matmul(ps_rt[:], w2bt[:], xb_b[b][:],
                                 start=True, stop=True)
                lt = setup.tile([E, K], FP32, tag=f"lt{b}", name=f"lt{b}")
                nc.scalar.copy(lt[:], ps_lt[:])
                lt_f.append(lt)
                rtb = setup.tile([E, K], BF16, tag=f"rtb{b}", name=f"rtb{b}")
                nc.vector.tensor_copy(rtb[:], ps_rt[:])
                rtb_b.append(rtb)

                # srb_j = alpha' * sum_e sgn_e rtb[e, j]; scale rhs rows
                # by exp(srb) in place
                for jh in range(2):
                    ps_srb = psM.tile([K // 2, 1], FP32, tag="srb")
                    nc.tensor.matmul(ps_srb[:],
                                     rtb[:, 128 * jh:128 * jh + 128],
                                     asgn_b, start=True, stop=True)
                    esrb = setup.tile([K // 2, 1], FP32, tag=f"esrb{b}{jh}",
                                      name=f"esrb{b}{jh}")
                    nc.scalar.activation(esrb[:], ps_srb[:], AF.Exp)
                    nc.vector.tensor_scalar(out=xto_s[b][jh],
                                            in0=xto_s[b][jh],
                                            scalar1=esrb[:], scalar2=None,
                                            op0=ALU.mult)

            # ---------- per-batch main pipeline ----------
            for b in range(BPC):
                ps_sc = [psS.tile([K // 2, K], FP32, tag=f"psS{jh}",
                                  name=f"psS{jh}") for jh in range(2)]
                for jh in range(2):
                    nc.tensor.matmul(ps_sc[jh][:], ident, bkkt[jh],
                                     start=True, stop=False,
                                     skip_group_check=True)
                sched = _SCHEDS[b]
                for i in range(K):
                    tj = trelu.tile([E, K], BF16, tag="tj")
                    eng = sched[i]
                    if eng == "D":
                        nc.vector.tensor_scalar(
                            out=tj[:], in0=rtb_b[b][:],
                            scalar1=lt_f[b][:, i:i + 1], scalar2=0.0,
                            op0=ALU.add, op1=ALU.max)
                    elif eng == "A":
                        nc.scalar.activation(
                            tj[:], rtb_b[b][:], AF.Relu,
                            bias=lt_f[b][:, i:i + 1], scale=1.0)
                    else:
                        nc.gpsimd.tensor_scalar(
                            out=tj[:], in0=rtb_b[b][:],
                            scalar1=lt_f[b][:, i:i + 1], scalar2=0.0,
                            op0=ALU.add, op1=ALU.max)
                    for jh in range(2):
                        nc.tensor.matmul(
                            ps_sc[jh][:, i:i + 1],
                            tj[:, 128 * jh:128 * jh + 128], sgn_b,
                            start=False, stop=(i == K - 1),
                            skip_group_check=True)

                # softmax numerator/denominator and epilogue
                pT = [setup.tile([K // 2, K], BF16, tag=f"pT{jh}",
                                 name=f"pT{jh}") for jh in range(2)]
                for jh in range(2):
                    nc.scalar.activation(pT[jh][:], ps_sc[jh][:], AF.Exp)

                t_ih = []
                for ih in range(2):
                    ps_num = psM.tile([K // 2, Wn + 1], FP32, tag="ps_num")
                    for jh in range(2):
                        nc.tensor.matmul(
                            ps_num[:], pT[jh][:, 128 * ih:128 * ih + 128],
                            xto_s[b][jh], start=(jh == 0), stop=(jh == 1))
                    rcol = setup.tile([K // 2, 1], FP32, tag=f"rcol{ih}",
                                      name=f"rcol{ih}")
                    nc.vector.reciprocal(rcol[:], ps_num[:, Wn:Wn + 1])
                    tt = setup.tile([K // 2, Wn], BF16, tag=f"tt{ih}",
                                    name=f"tt{ih}")
                    nc.scalar.activation(tt[:], ps_num[:, 0:Wn], AF.Tanh,
                                         scale=rcol[:])
                    t_ih.append(tt)

                ps_o = psM.tile([O, Wn], FP32, tag="ps_o")
                for ih in range(2):
                    nc.tensor.matmul(ps_o[:], fcw2t[ih], t_ih[ih][:],
                                     start=(ih == 0), stop=(ih == 1))
                ot = setup.tile([O, Wn], FP32, tag="ot")
                nc.scalar.activation(ot[:], ps_o[:], AF.Identity,
                                     bias=fcb2[:])
                nc.gpsimd.dma_start(out=d_out.ap()[b], in_=ot[:])

    nc.compile()
    return nc


_NC_CACHE = {}


def _get_program():
    if "nc" not in _NC_CACHE:
        _NC_CACHE["nc"] = _build_program()
    return _NC_CACHE["nc"]


def _host_prep(x, lin_w, lin_b, a, bias_kk, fc_w, fc_b):
    f32 = np.float32
    x = np.ascontiguousarray(x, f32)
    aa = (np.abs(a) * (1.0 - ALPHA)).astype(f32)
    sgn = np.sign(a).astype(f32)
    w1at = (lin_w[:, :Wn] * aa[:, None]).T.astype(NP_BF16)
    w2t = (lin_w[:, Wn:] * aa[:, None]).T
    bt = (lin_b * aa)[None, :]
    w2bt = np.concatenate([w2t, bt], 0).astype(NP_BF16)
    xb = np.concatenate([x, np.ones((B, 1, K), f32)], axis=1)
    xto = np.concatenate(
        [np.transpose(x, (0, 2, 1)),
         np.full((B, K, 1), 2.0, f32)], axis=2)        # (B, K, 65)
    # pack xto row-halves side by side: (B, 128, 130)
    xtp = np.concatenate([xto[:, :128, :], xto[:, 128:, :]], axis=2)
    sa = np.stack([sgn, 0.25 * sgn], axis=1)           # (E, 2)
    bkt = bias_kk.T.astype(f32)
    bki = np.concatenate([bkt[:128, :], bkt[128:, :],
                          np.eye(128, dtype=f32)], axis=1)  # (128, 640)
    fcw = (0.5 * fc_w).T                                # (256, 64)
    fcwp = np.concatenate([fcw[:128, :], fcw[128:, :]], axis=1)  # (128,128)
    shared = dict(
        w1at=np.ascontiguousarray(w1at),
        w2bt=np.ascontiguousarray(w2bt),
        sav=np.ascontiguousarray(sa.astype(NP_BF16)),
        bki=np.ascontiguousarray(bki.astype(NP_BF16)),
        fcw2t=np.ascontiguousarray(fcwp.astype(NP_BF16)),
        fcb2=np.ascontiguousarray(
            (fc_b + 0.5 * fc_w.sum(1)).reshape(O, 1).astype(f32)),
    )
    in_maps = []
    for c in range(N_CORES):
        m = dict(shared)
        m["xb"] = np.ascontiguousarray(
            xb[BPC * c:BPC * (c + 1)].astype(NP_BF16))
        m["xto"] = np.ascontiguousarray(
            xtp[BPC * c:BPC * (c + 1)].astype(NP_BF16))
        in_maps.append(m)
    return in_maps


def kernel(x, lin_w, lin_b, a, bias_kk, fc_w, fc_b, _trace=False):
    nc = _get_program()
    in_maps = _host_prep(np.asarray(x), np.asarray(lin_w), np.asarray(lin_b),
                         np.asarray(a), np.asarray(bias_kk),
                         np.asarray(fc_w), np.asarray(fc_b))
    res = run_bass_kernel_spmd(nc, in_maps, list(range(N_CORES)),
                               trace=_trace)
    out = np.empty((B, Wn, O), np.float32)
    for c in range(N_CORES):
        o = res.results[c]["outp"]          # (BPC, O, Wn)
        for i in range(BPC):
            out[BPC * c + i] = o[i].T
    if _trace:
        return out, res
    return out


# revision 9
# speedup vs baseline: 1.5369x; 1.0899x over previous
"""Trainium2 Bass kernel for nn_GatFeatDecoder (GAT-style decoder).

Reference computation per batch b (B=16, W=64, K=256, E=128, O=64):
    v = x[b].T                               (K, W)
    l = v @ W1.T ; r = v @ W2.T              (K, E) each
    e[i,j]  = sum_e a_e * LeakyReLU(l[i,e] + r[j,e] + lin_b[e]) + bias_kk[i,j]
    attn    = softmax_j(e)
    h       = sigmoid(attn @ v)              (K, W)
    out[b]  = h.T @ fc_w.T + fc_b            (W, O)

Kernel strategy (data-parallel, 2 batches per core on 8 cores, no
collectives):
  * Fold (1-alpha)*|a| into W1/W2/lin_b rows => z~ = (1-alpha)|a| z, and
    sum_e a_e*LeakyReLU(z) = sum_e sgn_e relu(z~) + alpha' * sum_e sgn_e z~
    with alpha' = alpha/(1-alpha).  The per-i part of the linear term is
    constant across j and cancels in softmax; the per-j part
    srb_j = alpha' * (sgn . rtb[:,j]) is folded MULTIPLICATIVELY into the
    attention rhs: exp(S + srb_j) = exp(S) * exp(srb_j), so the rows of
    [v | 2] are scaled by exp(srb_j) instead of adding srb pre-exp.
  * Relu tiles are generated PER QUERY NODE i: T_i[e, j] =
    relu(rtb[e, j] + lt[e, i]) with one fused tensor_scalar (DVE, 4x bf16
    mode), one ACT activation (Relu + per-partition bias), or one GPSIMD
    tensor_scalar - work split across all three engines by a static
    greedy load balancer (DVE ~127ns, ACT ~398ns, Pool ~451ns per tile).
  * The e-contraction uses the tile as STATIONARY weights: per (i, jh)
    one 1-row matmul with moving operand sgn [E,1] writes column i of
    the PSUM score tile S^T[j-half, i] directly in the [j, i] layout the
    softmax needs - no transpose, ~4ns of PE time per matmul.
  * bias_kk^T is pre-accumulated into the PSUM score tiles with one
    identity-weight matmul per (jh) before the column matmuls.
  * softmax without per-row max (logits bounded): P^T = exp(S^T).
    attn@v and the row-sum denominator come from one matmul per
    (i-half, j-half) with lhsT = P^T and rhs = exp(srb)*[v | 2]; then
    h = sigmoid(num/den) = 0.5*(tanh(num * (0.5/den)) + 1), realized as
    ACT Tanh with per-partition scale = reciprocal(2*sum exp), with the
    0.5/0.5 affine folded into the fc weights/bias on the host.
  * fc: out^T[o,w] = (0.5 fc_w)^T t + (fc_b + 0.5 sum_k fc_w).
  * DMA triggers ride on SP (HWDGE, zero engine cost) except the four
    startup-critical ones, which are spread across Pool/DVE/ACT while
    those engines are still idle.  Batch 1's input DMAs + projections are
    hoisted before batch 0's tile loop so the tile stream never stalls.
"""

import numpy as np
import ml_dtypes

import concourse.bass as bass
import concourse.bacc as bacc
import concourse.tile as tile
from concourse import mybir
from concourse.bass_utils import run_bass_kernel_spmd

ALPHA = 0.2
B, Wn, K, E, O = 16, 64, 256, 128, 64
N_CORES = 8
BPC = B // N_CORES  # batches per core

FP32 = mybir.dt.float32
BF16 = mybir.dt.bfloat16
AF = mybir.ActivationFunctionType
ALU = mybir.AluOpType
NP_BF16 = ml_dtypes.bfloat16


def _tile_engine_schedule(d0, a0, p0):
    """Static engine assignment for the 256 relu tiles of one batch:
    greedy finish-time balancing, initial loads = fixed per-batch work."""
    loads = {"D": float(d0), "A": float(a0), "P": float(p0)}
    cost = {"D": 127.0, "A": 398.0, "P": 451.0}
    sched = []
    for _ in range(K):
        e = min(loads, key=lambda k: loads[k] + cost[k])
        loads[e] += cost[e]
        sched.append(e)
    return sched


# batch 0: Pool paid ~1us for the w1at SWDGE trigger; batch 1 is clean
_SCHEDS = [_tile_engine_schedule(750, 2280, 1200),
           _tile_engine_schedule(750, 2280, 150)]


def _build_program():
    nc = bacc.Bacc("TRN2", target_bir_lowering=False, debug=False,
                   num_devices=N_CORES)

    # ---- per-core DRAM I/O (bf16 prepacked on host) ----
    d_xb = nc.dram_tensor("xb", [BPC, Wn + 1, K], BF16, kind="ExternalInput")
    # xto halves packed side by side: [j, (jh, w)] -> [128, 130]
    d_xto = nc.dram_tensor("xto", [BPC, K // 2, 2 * (Wn + 1)], BF16,
                           kind="ExternalInput")
    d_w1at = nc.dram_tensor("w1at", [Wn, E], BF16, kind="ExternalInput")
    d_w2bt = nc.dram_tensor("w2bt", [Wn + 1, E], BF16, kind="ExternalInput")
    d_sa = nc.dram_tensor("sav", [E, 2], BF16, kind="ExternalInput")
    # bias_kk^T halves + identity packed: [128, 256+256+128]
    d_bki = nc.dram_tensor("bki", [128, 2 * K + 128], BF16,
                           kind="ExternalInput")
    d_fcw2t = nc.dram_tensor("fcw2t", [K // 2, 2 * O], BF16,
                             kind="ExternalInput")
    d_fcb2 = nc.dram_tensor("fcb2", [O, 1], FP32, kind="ExternalInput")
    d_out = nc.dram_tensor("outp", [BPC, O, Wn], FP32, kind="ExternalOutput")

    with tile.TileContext(nc) as tc:
        with (
            tc.tile_pool(name="consts", bufs=1) as consts,
            tc.tile_pool(name="setup", bufs=2) as setup,
            tc.tile_pool(name="trelu", bufs=28) as trelu,
            tc.tile_pool(name="psP", bufs=1, space="PSUM") as psP,
            tc.tile_pool(name="psS", bufs=1, space="PSUM") as psS,
            tc.tile_pool(name="psM", bufs=1, space="PSUM") as psM,
        ):
            # ---------- constants ----------
            # startup-critical loads on engines that are idle at t=0
            w1at = consts.tile([Wn, E], BF16, tag="w1at")
            nc.gpsimd.dma_start(out=w1at[:], in_=d_w1at.ap())
            w2bt = consts.tile([Wn + 1, E], BF16, tag="w2bt")
            nc.sync.dma_start(out=w2bt[:], in_=d_w2bt.ap())
            sa = consts.tile([E, 2], BF16, tag="sa")
            nc.gpsimd.dma_start(out=sa[:], in_=d_sa.ap())
            sgn_b = sa[:, 0:1]
            asgn_b = sa[:, 1:2]
            bki = consts.tile([128, 2 * K + 128], BF16, tag="bki")
            nc.sync.dma_start(out=bki[:], in_=d_bki.ap())
            bkkt = [bki[:, 0:K], bki[:, K:2 * K]]
            ident = bki[:, 2 * K:2 * K + 128]
            fcw = consts.tile([K // 2, 2 * O], BF16, tag="fcw")
            nc.sync.dma_start(out=fcw[:], in_=d_fcw2t.ap())
            fcw2t = [fcw[:, 0:O], fcw[:, O:2 * O]]
            fcb2 = consts.tile([O, 1], FP32, tag="fcb2")
            nc.sync.dma_start(out=fcb2[:], in_=d_fcb2.ap())

            # ---------- per-batch setup (both batches up front) ----------
            xb_b, lt_f, rtb_b, xto_s = [], [], [], []
            for b in range(BPC):
                xb = setup.tile([Wn + 1, K], BF16, tag=f"xb{b}",
                                name=f"xb{b}")
                if b == 0:
                    nc.gpsimd.dma_start(out=xb[:], in_=d_xb.ap()[b])
                else:
                    nc.sync.dma_start(out=xb[:], in_=d_xb.ap()[b])
                xb_b.append(xb)
                xt = setup.tile([K // 2, 2 * (Wn + 1)], BF16, tag=f"xt{b}",
                                name=f"xt{b}")
                nc.sync.dma_start(out=xt[:], in_=d_xto.ap()[b])
                xto_s.append([xt[:, 0:Wn + 1], xt[:, Wn + 1:2 * (Wn + 1)]])

            for b in range(BPC):
                ps_lt = psP.tile([E, K], FP32, tag="ps_lt")
                nc.tensor.matmul(ps_lt[:], w1at[:], xb_b[b][0:Wn, :],
                                 start=True, stop=True)
                ps_rt = psP.tile([E, K], FP32, tag="ps_rt")
                nc.tensor.matmul(ps_rt[:], w2bt[:], xb_b[b][:],
                                 start=True, stop=True)
                lt = setup.tile([E, K], FP32, tag=f"lt{b}", name=f"lt{b}")
                nc.scalar.copy(lt[:], ps_lt[:])
                lt_f.append(lt)
                rtb = setup.tile([E, K], BF16, tag=f"rtb{b}", name=f"rtb{b}")
                nc.vector.tensor_copy(rtb[:], ps_rt[:])
                rtb_b.append(rtb)

                # srb_j = alpha' * sum_e sgn_e rtb[e, j]; scale rhs rows
                # by exp(srb) in place
                for jh in range(2):
                    ps_srb = psM.tile([K // 2, 1], FP32, tag="srb")
                    nc.tensor.matmul(ps_srb[:],
                                     rtb[:, 128 * jh:128 * jh + 128],
                                     asgn_b, start=True, stop=True)
                    esrb = setup.tile([K // 2, 1], FP32, tag=f"esrb{b}{jh}",
                                      name=f"esrb{b}{jh}")
                    nc.scalar.activation(esrb[:], ps_srb[:], AF.Exp)
                    nc.vector.tensor_scalar(out=xto_s[b][jh],
                                            in0=xto_s[b][jh],
                                            scalar1=esrb[:], scalar2=None,
                                            op0=ALU.mult)

            # ---------- per-batch main pipeline ----------
            for b in range(BPC):
                ps_sc = [psS.tile([K // 2, K], FP32, tag=f"psS{jh}",
                                  name=f"psS{jh}") for jh in range(2)]
                for jh in range(2):
                    nc.tensor.matmul(ps_sc[jh][:], ident, bkkt[jh],
                                     start=True, stop=False,
                                     skip_group_check=True)
                sched = _SCHEDS[b]
                for i in range(K):
                    tj = trelu.tile([E, K], BF16, tag="tj")
                    eng = sched[i]
                    if eng == "D":
                        nc.vector.tensor_scalar(
                            out=tj[:], in0=rtb_b[b][:],
                            scalar1=lt_f[b][:, i:i + 1], scalar2=0.0,
                            op0=ALU.add, op1=ALU.max)
                    elif eng == "A":
                        nc.scalar.activation(
                            tj[:], rtb_b[b][:], AF.Relu,
                            bias=lt_f[b][:, i:i + 1], scale=1.0)
                    else:
                        nc.gpsimd.tensor_scalar(
                            out=tj[:], in0=rtb_b[b][:],
                            scalar1=lt_f[b][:, i:i + 1], scalar2=0.0,
                            op0=ALU.add, op1=ALU.max)
                    for jh in range(2):
                        nc.tensor.matmul(
                            ps_sc[jh][:, i:i + 1],
                            tj[:, 128 * jh:128 * jh + 128], sgn_b,
                            start=False, stop=(i == K - 1),
                            skip_group_check=True)

                # softmax numerator/denominator and epilogue
                pT = [setup.tile([K // 2, K], BF16, tag=f"pT{jh}",
                                 name=f"pT{jh}") for jh in range(2)]
                for jh in range(2):
                    nc.scalar.activation(pT[jh][:], ps_sc[jh][:], AF.Exp)

                t_ih = []
                for ih in range(2):
                    ps_num = psM.tile([K // 2, Wn + 1], FP32, tag="ps_num")
                    for jh in range(2):
                        nc.tensor.matmul(
                            ps_num[:], pT[jh][:, 128 * ih:128 * ih + 128],
                            xto_s[b][jh], start=(jh == 0), stop=(jh == 1))
                    rcol = setup.tile([K // 2, 1], FP32, tag=f"rcol{ih}",
                                      name=f"rcol{ih}")
                    nc.vector.reciprocal(rcol[:], ps_num[:, Wn:Wn + 1])
                    tt = setup.tile([K // 2, Wn], BF16, tag=f"tt{ih}",
                                    name=f"tt{ih}")
                    nc.scalar.activation(tt[:], ps_num[:, 0:Wn], AF.Tanh,
                                         scale=rcol[:])
                    t_ih.append(tt)

                ps_o = psM.tile([O, Wn], FP32, tag="ps_o")
                for ih in range(2):
                    nc.tensor.matmul(ps_o[:], fcw2t[ih], t_ih[ih][:],
                                     start=(ih == 0), stop=(ih == 1))
                ot = setup.tile([O, Wn], FP32, tag="ot")
                nc.scalar.activation(ot[:], ps_o[:], AF.Identity,
                                     bias=fcb2[:])
                nc.sync.dma_start(out=d_out.ap()[b], in_=ot[:])

    nc.compile()
    return nc


_NC_CACHE = {}


def _get_program():
    if "nc" not in _NC_CACHE:
        _NC_CACHE["nc"] = _build_program()
    return _NC_CACHE["nc"]


def _host_prep(x, lin_w, lin_b, a, bias_kk, fc_w, fc_b):
    f32 = np.float32
    x = np.ascontiguousarray(x, f32)
    aa = (np.abs(a) * (1.0 - ALPHA)).astype(f32)
    sgn = np.sign(a).astype(f32)
    w1at = (lin_w[:, :Wn] * aa[:, None]).T.astype(NP_BF16)
    w2t = (lin_w[:, Wn:] * aa[:, None]).T
    bt = (lin_b * aa)[None, :]
    w2bt = np.concatenate([w2t, bt], 0).astype(NP_BF16)
    xb = np.concatenate([x, np.ones((B, 1, K), f32)], axis=1)
    xto = np.concatenate(
        [np.transpose(x, (0, 2, 1)),
         np.full((B, K, 1), 2.0, f32)], axis=2)        # (B, K, 65)
    # pack xto row-halves side by side: (B, 128, 130)
    xtp = np.concatenate([xto[:, :128, :], xto[:, 128:, :]], axis=2)
    sa = np.stack([sgn, 0.25 * sgn], axis=1)           # (E, 2)
    bkt = bias_kk.T.astype(f32)
    bki = np.concatenate([bkt[:128, :], bkt[128:, :],
                          np.eye(128, dtype=f32)], axis=1)  # (128, 640)
    fcw = (0.5 * fc_w).T                                # (256, 64)
    fcwp = np.concatenate([fcw[:128, :], fcw[128:, :]], axis=1)  # (128,128)
    shared = dict(
        w1at=np.ascontiguousarray(w1at),
        w2bt=np.ascontiguousarray(w2bt),
        sav=np.ascontiguousarray(sa.astype(NP_BF16)),
        bki=np.ascontiguousarray(bki.astype(NP_BF16)),
        fcw2t=np.ascontiguousarray(fcwp.astype(NP_BF16)),
        fcb2=np.ascontiguousarray(
            (fc_b + 0.5 * fc_w.sum(1)).reshape(O, 1).astype(f32)),
    )
    in_maps = []
    for c in range(N_CORES):
        m = dict(shared)
        m["xb"] = np.ascontiguousarray(
            xb[BPC * c:BPC * (c + 1)].astype(NP_BF16))
        m["xto"] = np.ascontiguousarray(
            xtp[BPC * c:BPC * (c + 1)].astype(NP_BF16))
        in_maps.append(m)
    return in_maps


def kernel(x, lin_w, lin_b, a, bias_kk, fc_w, fc_b, _trace=False):
    nc = _get_program()
    in_maps = _host_prep(np.asarray(x), np.asarray(lin_w), np.asarray(lin_b),
                         np.asarray(a), np.asarray(bias_kk),
                         np.asarray(fc_w), np.asarray(fc_b))
    res = run_bass_kernel_spmd(nc, in_maps, list(range(N_CORES)),
                               trace=_trace)
    out = np.empty((B, Wn, O), np.float32)
    for c in range(N_CORES):
        o = res.results[c]["outp"]          # (BPC, O, Wn)
        for i in range(BPC):
            out[BPC * c + i] = o[i].T
    if _trace:
        return out, res
    return out
